# revision 7
# baseline (speedup 1.0000x reference)
"""MoE kernel for Trainium2 (8 NeuronCores, expert-parallel).

Problem: nn_MoE_78151224918194
  hidden_states [4, 2048, 2048] f32 -> out [4, 2048, 2048] f32
  E=8 routed experts (top-2, softmax-renormalized), I=1408,
  plus a shared SwiGLU FFN with IS=2816.

Strategy:
  - Gate (softmax + top-2) computed on host with jax-on-CPU, exactly
    mirroring the reference ops, so expert selection matches bitwise.
  - Expert-parallel: core c runs expert c's FFN over the tokens routed to
    it (host-gathered, padded to C = max count rounded up to 256).
  - Shared FFN is token-parallel: core c also runs the shared expert
    (split into two I=1408 halves: jobs A and B) over token slice
    [c*1024, (c+1)*1024).
  - All matmuls run as fp32r (TF32-like) with weights stationary and
    tokens as the moving operand; activations are passed in pre-transposed
    [H, tokens] so the kernel needs no on-device transposes.
  - Host combine: y = concat(shared slices) then y[idx_e] += w_e * yr_e.
"""

import os
import numpy as np

import concourse.bacc as bacc
import concourse.mybir as mybir
import concourse.tile as tile
from concourse.bass_utils import run_bass_kernel_spmd

P = 128
H = 2048
I = 1408
E = 8
TOP_K = 2
IS = 2816
SHARED_SLICE = 1024
KH = H // P   # 16 k-tiles over H
KI = I // P   # 11 k-tiles over I
F32 = mybir.dt.float32
F32R = mybir.dt.float32r
TB_MAX = 1024
NSUB = 512

LAST_RESULTS = None  # BassKernelResults of the most recent device run
_BUILD_CACHE = {}


def _emit_ffn_job(nc, sbuf, psum, x_ap, wg_ap, wu_ap, wd_ap, out_ap, n_tok, jtag):
    """One SwiGLU FFN (I=1408) over n_tok tokens.

    x_ap:   DRAM [H, n_tok]   (tokens transposed)
    wg/wu:  DRAM [KI, P, KH, P]  (i-tile, partition(H), k-tile, I-cols)
    wd:     DRAM [KH, P, KI, P]  (m-tile, partition(I), i-tile, H-cols)
    out_ap: DRAM [H, n_tok]   out = ((silu(x@wg) * (x@wu)) @ wd).T
    """
    silu = mybir.ActivationFunctionType.Silu
    for b0 in range(0, n_tok, TB_MAX):
        TB = min(TB_MAX, n_tok - b0)
        nsubs = [(s, min(NSUB, TB - s)) for s in range(0, TB, NSUB)]
        bt = f"{jtag}b{b0}"

        x_tiles = []
        for k in range(KH):
            xt = sbuf.tile([P, TB], F32R, name=f"x{bt}k{k}", tag=f"x{k}", bufs=1)
            nc.sync.dma_start(xt[:], x_ap[k * P:(k + 1) * P, b0:b0 + TB])
            x_tiles.append(xt)

        act_tiles = []
        for i in range(KI):
            wg_sb = sbuf.tile([P, KH, P], F32R, name=f"wg{bt}i{i}", tag="wg", bufs=2)
            wu_sb = sbuf.tile([P, KH, P], F32R, name=f"wu{bt}i{i}", tag="wu", bufs=2)
            nc.sync.dma_start(wg_sb[:], wg_ap[i])
            nc.sync.dma_start(wu_sb[:], wu_ap[i])
            act = sbuf.tile([P, TB], F32R, name=f"act{bt}i{i}", tag=f"act{i}", bufs=1)
            for s, w in nsubs:
                pg = psum.tile([P, NSUB], F32, name=f"pg{bt}i{i}s{s}", tag="pg", bufs=2)
                pu = psum.tile([P, NSUB], F32, name=f"pu{bt}i{i}s{s}", tag="pu", bufs=2)
                for k in range(KH):
                    nc.tensor.matmul(
                        pg[:, :w], wg_sb[:, k],
                        x_tiles[k][:, s:s + w],
                        start=(k == 0), stop=(k == KH - 1))
                for k in range(KH):
                    nc.tensor.matmul(
                        pu[:, :w], wu_sb[:, k],
                        x_tiles[k][:, s:s + w],
                        start=(k == 0), stop=(k == KH - 1))
                tmp = sbuf.tile([P, NSUB], F32, name=f"tmp{bt}i{i}s{s}", tag="silu",
                                bufs=3)
                nc.scalar.activation(tmp[:, :w], pg[:, :w], silu)
                nc.vector.tensor_tensor(act[:, s:s + w], tmp[:, :w], pu[:, :w],
                                        mybir.AluOpType.mult)
            act_tiles.append(act)

        for m in range(KH):
            wd_sb = sbuf.tile([P, KI, P], F32R, name=f"wd{bt}m{m}", tag="wd", bufs=2)
            nc.sync.dma_start(wd_sb[:], wd_ap[m])
            for s, w in nsubs:
                po = psum.tile([P, NSUB], F32, name=f"po{bt}m{m}s{s}", tag="po",
                               bufs=2)
                for i in range(KI):
                    nc.tensor.matmul(
                        po[:, :w], wd_sb[:, i],
                        act_tiles[i][:, s:s + w],
                        start=(i == 0), stop=(i == KI - 1))
                ot = sbuf.tile([P, NSUB], F32, name=f"ot{bt}m{m}s{s}", tag="ot",
                               bufs=3)
                nc.vector.tensor_copy(ot[:, :w], po[:, :w])
                nc.sync.dma_start(out_ap[m * P:(m + 1) * P, b0 + s:b0 + s + w],
                                  ot[:, :w])


def _build(C, reps=1, loop=0):
    nc = bacc.Bacc(trn_type="TRN2", target_bir_lowering=False, debug=False)
    W_SHAPE = [KI, P, KH, P]
    D_SHAPE = [KH, P, KI, P]
    xr = nc.dram_tensor("xr", [H, C], F32R, kind="ExternalInput")
    xs = nc.dram_tensor("xs", [H, SHARED_SLICE], F32R, kind="ExternalInput")
    w_in = {}
    for nm in ("rg", "ru", "ag", "au", "bg", "bu"):
        w_in[nm] = nc.dram_tensor(nm, W_SHAPE, F32R, kind="ExternalInput")
    for nm in ("rd", "ad", "bd"):
        w_in[nm] = nc.dram_tensor(nm, D_SHAPE, F32R, kind="ExternalInput")
    yr = nc.dram_tensor("yr", [H, C], F32, kind="ExternalOutput")
    ya = nc.dram_tensor("ya", [H, SHARED_SLICE], F32, kind="ExternalOutput")
    yb = nc.dram_tensor("yb", [H, SHARED_SLICE], F32, kind="ExternalOutput")

    with tile.TileContext(nc) as tc:
        with (
            tc.tile_pool(name="sbuf", bufs=2) as sbuf,
            tc.tile_pool(name="psum", bufs=2, space="PSUM") as psum,
        ):
            def body():
                for r in range(reps):
                    _emit_ffn_job(nc, sbuf, psum, xr.ap(), w_in["rg"].ap(),
                                  w_in["ru"].ap(), w_in["rd"].ap(), yr.ap(), C,
                                  f"r{r}_")
                    _emit_ffn_job(nc, sbuf, psum, xs.ap(), w_in["ag"].ap(),
                                  w_in["au"].ap(), w_in["ad"].ap(), ya.ap(),
                                  SHARED_SLICE, f"a{r}_")
                    _emit_ffn_job(nc, sbuf, psum, xs.ap(), w_in["bg"].ap(),
                                  w_in["bu"].ap(), w_in["bd"].ap(), yb.ap(),
                                  SHARED_SLICE, f"b{r}_")

            if loop:
                with tc.For_i(0, loop, 1):
                    body()
            else:
                body()
    nc.compile()
    return nc


def _get_nc(C, reps=1, loop=0):
    key = (C, reps, loop)
    if key not in _BUILD_CACHE:
        _BUILD_CACHE[key] = _build(C, reps, loop)
    return _BUILD_CACHE[key]


def _gate_host(x, gate_w):
    """Softmax + top-2 + renormalize, mirroring the jax reference on CPU."""
    try:
        import jax
        import jax.numpy as jnp
        cpu = jax.devices("cpu")[0]
        with jax.default_device(cpu):
            logits = jnp.asarray(x) @ jnp.asarray(gate_w).T
            scores = jax.nn.softmax(logits, axis=-1)
            topk_w, topk_idx = jax.lax.top_k(scores, TOP_K)
            topk_w = topk_w / (jnp.sum(topk_w, axis=-1, keepdims=True) + 1e-20)
        return np.asarray(topk_w), np.asarray(topk_idx)
    except Exception:
        logits = x @ gate_w.T
        m = logits.max(axis=-1, keepdims=True)
        ex = np.exp(logits - m)
        scores = ex / ex.sum(axis=-1, keepdims=True)
        order = np.argsort(-scores, axis=-1, kind="stable")
        topk_idx = order[:, :TOP_K]
        topk_w = np.take_along_axis(scores, topk_idx, axis=-1)
        topk_w = topk_w / (topk_w.sum(axis=-1, keepdims=True) + 1e-20)
        return topk_w.astype(np.float32), topk_idx.astype(np.int32)


def _wlayout_ud(w):
    # [H, I_like] -> [KI', P(H), KH, P(I)]  (stationary tiles for up/gate)
    ki = w.shape[1] // P
    return np.ascontiguousarray(
        w.reshape(KH, P, ki, P).transpose(2, 1, 0, 3))


def _wlayout_down(w):
    # [I_like, H] -> [KH, P(I), KI', P(H)]
    ki = w.shape[0] // P
    return np.ascontiguousarray(
        w.reshape(ki, P, KH, P).transpose(2, 1, 0, 3))


def _prepare(hidden_states, gate_w, we_gate, we_up, we_down,
             ws_gate, ws_up, ws_down):
    B, S, h = hidden_states.shape
    x = np.ascontiguousarray(hidden_states.reshape(-1, h))  # [T, H]

    topk_w, topk_idx = _gate_host(x, gate_w)

    idx_lists, w_lists = [], []
    for e in range(E):
        mask = (topk_idx == e)
        idx = np.nonzero(mask.any(axis=1))[0]
        we = np.where(mask, topk_w, 0.0).sum(axis=1)[idx].astype(np.float32)
        idx_lists.append(idx)
        w_lists.append(we)
    max_cnt = max(len(ix) for ix in idx_lists)
    C = max(256, ((max_cnt + 255) // 256) * 256)

    # Shared-expert weights (same arrays for every core).
    ag = _wlayout_ud(ws_gate[:, :I])
    bg = _wlayout_ud(ws_gate[:, I:])
    au = _wlayout_ud(ws_up[:, :I])
    bu = _wlayout_ud(ws_up[:, I:])
    ad = _wlayout_down(ws_down[:I])
    bd = _wlayout_down(ws_down[I:])

    in_maps = []
    for c in range(E):
        idx = idx_lists[c]
        xr = np.zeros((H, C), dtype=np.float32)
        xr[:, :len(idx)] = x[idx].T
        xs = np.ascontiguousarray(
            x[c * SHARED_SLICE:(c + 1) * SHARED_SLICE].T)
        in_maps.append({
            "xr": xr, "xs": xs,
            "rg": _wlayout_ud(we_gate[c]),
            "ru": _wlayout_ud(we_up[c]),
            "rd": _wlayout_down(we_down[c]),
            "ag": ag, "au": au, "ad": ad,
            "bg": bg, "bu": bu, "bd": bd,
        })
    return in_maps, idx_lists, w_lists, C


def _combine(results, idx_lists, w_lists, T):
    y = np.empty((T, H), dtype=np.float32)
    for c in range(E):
        sh = results[c]["ya"] + results[c]["yb"]
        y[c * SHARED_SLICE:(c + 1) * SHARED_SLICE] = sh.T
    for c in range(E):
        idx = idx_lists[c]
        y[idx] += w_lists[c][:, None] * results[c]["yr"][:, :len(idx)].T
    return y


def kernel(hidden_states, gate_w, we_gate, we_up, we_down,
           ws_gate, ws_up, ws_down):
    global LAST_RESULTS
    B, S, h = hidden_states.shape
    in_maps, idx_lists, w_lists, C = _prepare(
        hidden_states, gate_w, we_gate, we_up, we_down,
        ws_gate, ws_up, ws_down)

    nc = _get_nc(C)

    trace_env = os.environ.get("MOE_TRACE", "")
    kwargs = {}
    if trace_env:
        kwargs["trace"] = True
        kwargs["trace_cores"] = [int(t) for t in trace_env.split(",")] \
            if trace_env != "1" else [0]
    res = run_bass_kernel_spmd(nc, in_maps, core_ids=list(range(E)), **kwargs)
    LAST_RESULTS = res

    y = _combine(res.results, idx_lists, w_lists, B * S)
    return y.reshape(B, S, h)
